# revision 35
# baseline (speedup 1.0000x reference)
"""Trainium2 Bass kernel for nn_Confidence_Score (gnn_message_passing).

Math: with S_g = sum of x over nodes of graph g and n_g = node count,
every node of graph g has identical activations:
    h1_g = relu(S_g @ W1 + b1)
    h2_g = relu((n_g * h1_g) @ W2 + b2)
    c_g  = h2_g @ Wc + bc ;  out_node = sp/(1+sp), sp = softplus(c_g)
Kernel: (pass 1) segment-sum x -> S via one-hot matmuls, while also
building the transposed one-hot A_T[g, node] in bulk (ones-broadcast
matmul + is_eq); (MLP) tiny per-graph network; (pass 2) out = og.T @ A_T
as 512-wide matmuls with og split into exact bf16 hi+lo parts.

PE runs bf16 everywhere data allows: one-hots are exact in bf16, and
x is split x = x_hi + x_lo (both bf16, exact to ~2^-18 rel) so one
N=256 moving matmul per 128-node chunk accumulates S_hi|S_lo in PSUM.

Sharding: graph-aligned contiguous node ranges, balanced by node count,
one range per core (8 cores); weights replicated; no collectives.
"""

import os
import sys

for _p in ("/root/.axon_site", "/root/.axon_site/_ro/trn_rl_repo",
           "/root/.axon_site/_ro/pypackages", "/opt/trn_rl_repo"):
    if os.path.isdir(_p) and _p not in sys.path:
        sys.path.append(_p)

import numpy as np

N_CORES = 8
D = 128
H = 256
G_TOTAL = 512
G_PAD = 72        # max local graphs per core (actual ~66)
CHUNK = 128       # nodes per aggregation matmul
XB = 10           # chunks per x DMA
ROW = 2 * D + 2   # x_hi | x_lo | bt | pad, bf16
OB = 512          # nodes per expansion matmul / A_T-gen block

# packed f32 const layout (columns in "cpk" [128, CPK]):
C_ID = 0            # ident [128,128]
C_W1 = 128          # w1 [128,256]
C_W2A = 384         # w2 rows 0-127 [128,256]
C_W2B = 640         # w2 rows 128-255 [128,256]
C_WC = 896          # wc as 2 cols: [0:128] and [128:256]
C_NC = 898          # ncol [G_PAD,1]
C_BC = 899          # bccol [G_PAD,1]
C_IO = 900          # iota column 0..127 [128,1]
C_B1 = 901          # b1 broadcast [G_PAD,256]
C_B2 = 1157         # b2 broadcast [G_PAD,256]
CPK = 1413

_CACHE = {}


def _build(nodes_pad):
    """Build + compile the single-core Bass program (shapes uniform across cores)."""
    from contextlib import ExitStack

    import concourse.bacc as bacc
    import concourse.mybir as mybir
    import concourse.tile as tile

    f32 = mybir.dt.float32
    bf16 = mybir.dt.bfloat16
    AF = mybir.ActivationFunctionType
    OP = mybir.AluOpType

    n_chunks = nodes_pad // CHUNK
    assert n_chunks % XB == 0
    n_ob = nodes_pad // OB

    nc = bacc.Bacc("TRN2", target_bir_lowering=False, debug=False)

    xb_d = nc.dram_tensor("xb", [nodes_pad, ROW], bf16, kind="ExternalInput").ap()
    bta_d = nc.dram_tensor("btall", [1, nodes_pad], bf16, kind="ExternalInput").ap()
    cpk_d = nc.dram_tensor("cpk", [128, CPK], f32, kind="ExternalInput").ap()
    io_d = nc.dram_tensor("iotab", [128, G_PAD], bf16, kind="ExternalInput").ap()
    out_d = nc.dram_tensor("out", [n_ob, OB], f32, kind="ExternalOutput").ap()

    # host pre-shuffles xb so each (group, partition) segment is contiguous
    xb_groups = xb_d.rearrange("(g p j) d -> g p (j d)", p=CHUNK, j=XB)

    with tile.TileContext(nc) as tc, ExitStack() as ctx:
        const = ctx.enter_context(tc.tile_pool(name="const", bufs=1))
        store = ctx.enter_context(tc.tile_pool(name="store", bufs=1))
        ps_s = ctx.enter_context(tc.tile_pool(name="ps_s", bufs=1, space="PSUM"))

        cpk = const.tile([128, CPK], f32)
        nc.scalar.dma_start(cpk[:], cpk_d[:])
        ident = cpk[:, C_ID:C_ID + 128]
        w1_s = cpk[:, C_W1:C_W1 + H]
        w2a = cpk[:, C_W2A:C_W2A + H]
        w2b = cpk[:, C_W2B:C_W2B + H]
        wca = cpk[:, C_WC:C_WC + 1]
        wcb = cpk[:, C_WC + 1:C_WC + 2]
        ncs = cpk[0:G_PAD, C_NC:C_NC + 1]
        bcs = cpk[0:G_PAD, C_BC:C_BC + 1]
        io72 = cpk[0:G_PAD, C_IO:C_IO + 1]
        b1s = cpk[0:G_PAD, C_B1:C_B1 + H]
        b2s = cpk[0:G_PAD, C_B2:C_B2 + H]

        iota_b = const.tile([128, G_PAD], bf16)
        nc.scalar.dma_start(iota_b[:], io_d[:])
        btb = store.tile([G_PAD, nodes_pad], bf16)

        at_sb = store.tile([G_PAD, nodes_pad], bf16)
        es2 = store.tile([96, ((n_ob + 2) // 3) * OB], f32)
        s_ps = ps_s.tile([G_PAD, 2 * D], f32)

        # ---- pass 1: segment-sum (hi|lo bf16) + A_T generation ----
        TBW = 512
        n_tb = nodes_pad // TBW
        n_tb_per_g = -(-n_tb // (n_chunks // XB - 2))
        with (
            tc.tile_pool(name="xp", bufs=8) as xpool,
            tc.tile_pool(name="ap", bufs=8) as apool,
        ):
            for g in range(n_chunks // XB):
                xt = xpool.tile([CHUNK, XB * ROW], bf16)
                eng = nc.sync if g % 2 == 0 else nc.scalar
                eng.dma_start(xt[:], xb_groups[g])
                half = nodes_pad // 2
                if g == 0:
                    nc.scalar.dma_start(
                        btb[:, half:],
                        bta_d[0:1, half:].to_broadcast((G_PAD, nodes_pad - half)))
                if g == 1:
                    nc.sync.dma_start(
                        btb[:, 0:half],
                        bta_d[0:1, 0:half].to_broadcast((G_PAD, half)))
                for j in range(XB):
                    c = g * XB + j
                    a = apool.tile([CHUNK, G_PAD], bf16)
                    # [zero, bf16(bt)] pair bitcast to one f32 == f32(bt) exactly
                    nc.vector.tensor_scalar(
                        a[:], iota_b[:],
                        xt[:, j * ROW + 2 * D:j * ROW + 2 * D + 2].bitcast(f32),
                        None, op0=OP.is_equal,
                    )
                    nc.tensor.matmul(
                        s_ps[:], lhsT=a[:], rhs=xt[:, j * ROW:j * ROW + 2 * D],
                        start=(c == 0), stop=(c == n_chunks - 1),
                    )
                # A_T blocks, interleaved so they hide under the DMA window
                if g >= 2:
                    lo = (g - 2) * n_tb_per_g
                    hi = n_tb if g == n_chunks // XB - 1 else lo + n_tb_per_g
                    for tb in range(lo, min(hi, n_tb)):
                        nc.vector.tensor_scalar(
                            at_sb[:, tb * TBW:(tb + 1) * TBW],
                            btb[:, tb * TBW:(tb + 1) * TBW], io72[:],
                            None, op0=OP.is_equal,
                        )

        # ---- per-graph MLP ----
        with (
            tc.tile_pool(name="mlp", bufs=1) as mlp,
            tc.tile_pool(name="ps_m", bufs=2, space="PSUM") as ps_m,
        ):
            s_lo = mlp.tile([G_PAD, D], f32)
            nc.vector.tensor_copy(s_lo[:], s_ps[:, D:2 * D])
            s_sb = mlp.tile([G_PAD, D], f32)
            nc.vector.tensor_tensor(s_sb[:], s_ps[:, 0:D], s_lo[:], op=OP.add)
            st_ps = ps_m.tile([D, G_PAD], f32, tag="tps")
            nc.tensor.transpose(st_ps[:], s_sb[:], ident[0:G_PAD, 0:G_PAD])
            st_sb = mlp.tile([D, G_PAD], f32)
            nc.vector.tensor_copy(st_sb[:], st_ps[:])

            h1_ps = ps_m.tile([G_PAD, H], f32, tag="mm")
            nc.tensor.matmul(h1_ps[:], lhsT=st_sb[:], rhs=w1_s[:], start=True, stop=True)
            h1 = mlp.tile([G_PAD, H], f32)
            nc.vector.tensor_tensor(h1[:], h1_ps[:], b1s[:], op=OP.add)
            nc.vector.tensor_scalar_max(h1[:], h1[:], 0.0)
            nc.vector.tensor_scalar_mul(h1[:], h1[:], ncs[:])

            h2_ps = ps_m.tile([G_PAD, H], f32, tag="mm")
            for kk in range(2):
                tp = ps_m.tile([128, G_PAD], f32, tag="tps")
                nc.tensor.transpose(
                    tp[:], h1[:, kk * 128:(kk + 1) * 128], ident[0:G_PAD, 0:G_PAD]
                )
                tsb = mlp.tile([128, G_PAD], f32, tag=f"tsb{kk}")
                nc.vector.tensor_copy(tsb[:], tp[:])
                nc.tensor.matmul(
                    h2_ps[:], lhsT=tsb[:], rhs=(w2a[:] if kk == 0 else w2b[:]),
                    start=(kk == 0), stop=(kk == 1),
                )
            h2 = mlp.tile([G_PAD, H], f32)
            nc.vector.tensor_tensor(h2[:], h2_ps[:], b2s[:], op=OP.add)
            nc.vector.tensor_scalar_max(h2[:], h2[:], 0.0)

            c_ps = ps_m.tile([G_PAD, 1], f32, tag="mm")
            for kk in range(2):
                tp = ps_m.tile([128, G_PAD], f32, tag="tps")
                nc.tensor.transpose(
                    tp[:], h2[:, kk * 128:(kk + 1) * 128], ident[0:G_PAD, 0:G_PAD]
                )
                tsb = mlp.tile([128, G_PAD], f32, tag=f"usb{kk}")
                nc.vector.tensor_copy(tsb[:], tp[:])
                nc.tensor.matmul(
                    c_ps[:], lhsT=tsb[:], rhs=(wca[:] if kk == 0 else wcb[:]),
                    start=(kk == 0), stop=(kk == 1),
                )

            # sp = softplus(c+bc) = relu(c) + log1p(exp(-|c|)); out = sp/(1+sp)
            cc = mlp.tile([G_PAD, 1], f32)
            nc.vector.tensor_scalar_add(cc[:], c_ps[:], bcs[:])
            negc = mlp.tile([G_PAD, 1], f32)
            nc.vector.tensor_scalar_mul(negc[:], cc[:], -1.0)
            nab = mlp.tile([G_PAD, 1], f32)
            nc.vector.tensor_tensor(nab[:], cc[:], negc[:], op=OP.min)
            ex = mlp.tile([G_PAD, 1], f32)
            nc.scalar.activation(ex[:], nab[:], AF.Exp)
            ex1 = mlp.tile([G_PAD, 1], f32)
            nc.vector.tensor_scalar_add(ex1[:], ex[:], 1.0)
            lg = mlp.tile([G_PAD, 1], f32)
            nc.scalar.activation(lg[:], ex1[:], AF.Ln)
            rl = mlp.tile([G_PAD, 1], f32)
            nc.vector.tensor_scalar_max(rl[:], cc[:], 0.0)
            sp = mlp.tile([G_PAD, 1], f32)
            nc.vector.tensor_tensor(sp[:], rl[:], lg[:], op=OP.add)
            t1 = mlp.tile([G_PAD, 1], f32)
            nc.vector.tensor_scalar_add(t1[:], sp[:], 1.0)
            rcp = mlp.tile([G_PAD, 1], f32)
            nc.vector.reciprocal(rcp[:], t1[:])
            og = mlp.tile([G_PAD, 1], f32)
            nc.vector.tensor_scalar(
                og[:], rcp[:], -1.0, 1.0, op0=OP.mult, op1=OP.add
            )
            # exact bf16 hi/lo split of og
            ogh = mlp.tile([G_PAD, 1], bf16)
            nc.vector.tensor_copy(ogh[:], og[:])
            oghf = mlp.tile([G_PAD, 1], f32)
            nc.vector.tensor_copy(oghf[:], ogh[:])
            oglf = mlp.tile([G_PAD, 1], f32)
            nc.vector.tensor_tensor(oglf[:], og[:], oghf[:], op=OP.subtract)
            ogl = mlp.tile([G_PAD, 1], bf16)
            nc.vector.tensor_copy(ogl[:], oglf[:])
            zz = mlp.tile([G_PAD, 32], f32)
            nc.vector.memset(zz[:], 0.0)
            ogrh = const.tile([G_PAD, 32], bf16)
            nc.vector.tensor_scalar(ogrh[:], zz[:], oghf[:], None, op0=OP.add)
            ogrl = const.tile([G_PAD, 32], bf16)
            nc.vector.tensor_scalar(ogrl[:], zz[:], oglf[:], None, op0=OP.add)

        # ---- pass 2: out = og.T @ A_T, 512 nodes per matmul ----
        # block b (= r*NQ + q) -> bank-tile q, partition band 32*r
        NQ = (n_ob + 2) // 3
        with tc.tile_pool(name="ps_e", bufs=4, space="PSUM") as ps_e:
            for q in range(NQ):
                e_ps = ps_e.tile([96, OB], f32)
                for r in range(3):
                    b = r * NQ + q
                    if b >= n_ob:
                        continue
                    for w, ogx in ((0, ogrh), (1, ogrl)):
                        nc.tensor.matmul(
                            e_ps[32 * r:32 * r + 32, :], lhsT=ogx[:],
                            rhs=at_sb[:, b * OB:(b + 1) * OB],
                            start=(w == 0), stop=(w == 1),
                        )
                dst = es2[:, q * OB:(q + 1) * OB]
                if q % 2 == 0:
                    nc.vector.tensor_copy(dst, e_ps[:])
                else:
                    nc.scalar.copy(dst, e_ps[:])
            for r in range(3):
                nb = min(NQ, n_ob - r * NQ)
                if nb <= 0:
                    continue
                nc.sync.dma_start(
                    out_d[r * NQ:r * NQ + nb, :].rearrange("a i -> (a i)"),
                    es2[32 * r:32 * r + 1, 0:nb * OB],
                )

    nc.compile()
    return nc


def _shard(batch):
    """Graph-aligned split of nodes across cores, balanced by node count."""
    n = batch.shape[0]
    counts = np.bincount(batch, minlength=G_TOTAL).astype(np.int64)
    bounds = np.concatenate([[0], np.cumsum(counts)])
    gsplit = [0]
    for k in range(1, N_CORES):
        t = k * n // N_CORES
        g = int(np.searchsorted(bounds, t))
        if g > 0 and abs(int(bounds[g - 1]) - t) < abs(int(bounds[g]) - t):
            g -= 1
        g = min(max(g, gsplit[-1]), G_TOTAL)
        gsplit.append(g)
    gsplit.append(G_TOTAL)
    return counts, bounds, gsplit


def kernel(**inputs):
    import ml_dtypes
    from concourse.bass_utils import run_bass_kernel_spmd

    bf16 = ml_dtypes.bfloat16
    x = np.ascontiguousarray(np.asarray(inputs["x"], dtype=np.float32))
    batch = np.asarray(inputs["batch"]).astype(np.int64)
    W1 = np.asarray(inputs["W1"], dtype=np.float32)
    b1 = np.asarray(inputs["b1"], dtype=np.float32)
    W2 = np.asarray(inputs["W2"], dtype=np.float32)
    b2 = np.asarray(inputs["b2"], dtype=np.float32)
    Wc = np.asarray(inputs["Wc"], dtype=np.float32).reshape(H, 1)
    bc = np.asarray(inputs["bc"], dtype=np.float32).reshape(1)

    n = batch.shape[0]
    counts, bounds, gsplit = _shard(batch)
    node_cnt = [int(bounds[gsplit[k + 1]] - bounds[gsplit[k]]) for k in range(N_CORES)]
    pad_unit = np.lcm(CHUNK * XB, OB)  # DMA-group and expansion-block aligned
    nodes_pad = int(-(-max(node_cnt) // pad_unit) * pad_unit)
    assert nodes_pad % OB == 0
    assert max(gsplit[k + 1] - gsplit[k] for k in range(N_CORES)) <= G_PAD

    key = nodes_pad
    if key not in _CACHE:
        _CACHE[key] = _build(nodes_pad)
    nc = _CACHE[key]

    cpk = np.zeros((128, CPK), dtype=np.float32)
    cpk[:, C_ID:C_ID + 128] = np.eye(128, dtype=np.float32)
    cpk[:, C_W1:C_W1 + H] = W1
    cpk[:, C_W2A:C_W2A + H] = W2[0:128]
    cpk[:, C_W2B:C_W2B + H] = W2[128:256]
    cpk[:, C_WC] = Wc[0:128, 0]
    cpk[:, C_WC + 1] = Wc[128:256, 0]
    cpk[:, C_BC] = bc[0]
    cpk[:, C_IO] = np.arange(128, dtype=np.float32)
    cpk[0:G_PAD, C_B1:C_B1 + H] = b1
    cpk[0:G_PAD, C_B2:C_B2 + H] = b2

    n_groups = nodes_pad // (CHUNK * XB)
    in_maps = []
    for k in range(N_CORES):
        gs, ge = gsplit[k], gsplit[k + 1]
        ns, ne = int(bounds[gs]), int(bounds[ge])
        cnt = ne - ns
        bt = np.full(nodes_pad, G_PAD - 1, dtype=np.float32)
        bt[:cnt] = (batch[ns:ne] - gs).astype(np.float32)
        xh = x[ns:ne].astype(bf16)
        xl = (x[ns:ne] - xh.astype(np.float32)).astype(bf16)
        xbp = np.zeros((nodes_pad, ROW), dtype=bf16)
        xbp[:cnt, :D] = xh
        xbp[:cnt, D:2 * D] = xl
        xbp[:, 2 * D + 1] = bt.astype(bf16)  # high half of an f32 via bitcast
        # shuffle to (group, partition, chunk-in-group, row) DMA order
        xbp = np.ascontiguousarray(
            xbp.reshape(n_groups, XB, CHUNK, ROW).transpose(0, 2, 1, 3)
        ).reshape(nodes_pad, ROW)
        cpkk = cpk.copy()
        cpkk[gsplit[k + 1] - gs:G_PAD, C_NC] = 0.0
        cpkk[0:ge - gs, C_NC] = counts[gs:ge].astype(np.float32)
        in_maps.append({
            "xb": xbp,
            "btall": np.ascontiguousarray(bt.astype(bf16).reshape(1, nodes_pad)),
            "cpk": cpkk,
            "iotab": np.ascontiguousarray(
                np.broadcast_to(np.arange(G_PAD, dtype=np.float32),
                                (128, G_PAD)).astype(bf16)),
        })

    res = run_bass_kernel_spmd(nc, in_maps, core_ids=list(range(N_CORES)))
    outs = []
    for k in range(N_CORES):
        o = res.results[k]["out"].reshape(-1)
        outs.append(o[: node_cnt[k]])
    return np.concatenate(outs).reshape(n, 1).astype(np.float32)
